# revision 37
# baseline (speedup 1.0000x reference)
"""Trainium2 Bass kernel for the GNN neighbor-aggregation module.

Computation (per row r with K=32 neighbors, D=64):
    scores[r,k]  = sum_d rel[r,k,d] * user[r%B, d]
    w[r,:]       = softmax(scores[r,:])                (no max-subtract; |score|<60)
    agg[r,d]     = (1/K) sum_k nei[r,k,d] * w[r,k] * norms[r,k]
    out[r,:]     = relu((self[r,:] + agg[r,:]) @ W.T + b)

Sharding: pure data parallelism over rows across 8 cores (8192 rows/core).
W, b and user_embeddings are replicated (user index = r mod 4096, and
8192 % 4096 == 0, so local user indexing is identical on every shard).

Raw-bass implementation (this walrus build rejects instructions carrying
embedded multi-sem waits, which rules out TileContext): explicit semaphores,
standalone wait_ge instructions, double-buffered SBUF/PSUM, per-engine
programs via nc.Block().

Precision: neighbor_relations / neighbor_vectors / user_embeddings are cast
to bf16 on the host (halves their HBM traffic, enables the DVE 2x tensor-
tensor mode and single-pass PE matmuls).  Everything downstream of the
softmax numerator stays fp32.  Verified end-to-end rel_l2 vs the fp32
reference: 1.1e-04.

Per 128-row tile:
  DVE : scores = segmented-reduce(rel * user_bcast); softmax small ops;
        32x32 block-transpose of the weights; scatter into a block-diagonal
        [128,128] bf16 tile; x^T = agg^T + self^T.
  ACT : exp (with fused row-sum accumulator), PSUM->SBUF copies, relu+bias.
  PE  : 32 block-diagonal bf16 matmuls (lhsT = nei tile [128=(c,k), 64d],
        rhs = 4 block-diag weight columns) producing agg^T[d, r] directly
        in PSUM; self^T via PE transpose; y^T = W^T.T @ x^T; transpose back.
  DMA : self/norms/user fully preloaded at startup; per tile only
        rel (SP ring), 4 nei c-block transfers split SP/POOL-SWDGE rings,
        and the output store (SP).
"""

import numpy as np

import concourse.bass as bass
import concourse.mybir as mybir

fp32 = mybir.dt.float32
bf16 = mybir.dt.bfloat16
AF = mybir.ActivationFunctionType
ALU = mybir.AluOpType

R, D, B, K = 65536, 64, 4096, 32
NCORES = 8
R_LOC = R // NCORES  # 8192 rows per core
P = 128              # rows per tile


def build_nc(r_loc: int = R_LOC):
    nc = bass.Bass("TRN2", target_bir_lowering=False, debug=False)

    self_d = nc.dram_tensor("self_v", [r_loc, D], fp32, kind="ExternalInput")
    nv_d = nc.dram_tensor("nei_v", [r_loc * K, D], bf16, kind="ExternalInput")
    nr_d = nc.dram_tensor("nei_r", [r_loc * K, D], bf16, kind="ExternalInput")
    nn_d = nc.dram_tensor("nei_n", [r_loc * K], fp32, kind="ExternalInput")
    user_d = nc.dram_tensor("user", [B, D], bf16, kind="ExternalInput")
    w_d = nc.dram_tensor("W", [D, D], fp32, kind="ExternalInput")
    b_d = nc.dram_tensor("b", [D], fp32, kind="ExternalInput")
    out_d = nc.dram_tensor("out", [r_loc, D], fp32, kind="ExternalOutput")

    T = r_loc // P       # number of 128-row tiles
    UB = min(B, r_loc) // P  # distinct 128-row user blocks

    # DRAM views
    rel_rows = nr_d.ap().rearrange("(r k) d -> r (k d)", k=K)    # [r_loc, 2048]
    # nei matmul layout: nei[32c+k, 64j+d] = nv[(t*128+32c+j)*K + k, d],
    # one DMA per 32-partition block c with an affine [k, j, d] AP.
    nv_g = nv_d.ap().rearrange("(t c j k) d -> t c k j d", c=4, j=32, k=K)
    # preload views: all tiles side by side in the free dimension
    self_cols = self_d.ap().rearrange("(t p) d -> p t d", p=P)   # [128, T, 64]
    nn_cols = nn_d.ap().rearrange("(t p k) -> p t k", p=P, k=K)  # [128, T, 32]
    user_cols = user_d.ap().rearrange("(m p) d -> p m d", p=P)   # [128, B/128, 64]

    a = nc.alloc_sbuf_tensor
    rel = [a(f"rel{i}", [P, K * D], bf16) for i in range(3)]
    nei = [a(f"nei{i}", [P, K * D], bf16) for i in range(6)]
    prod = a("prod", [P, K * D], bf16)
    self_all = a("self_all", [P, T * D], fp32)
    norm_all = a("norm_all", [P, T * K], fp32)
    user_all = a("user_all", [P, UB * D], bf16)
    scores = [a(f"scores{i}", [P, K], fp32) for i in range(2)]
    # softmax tail is processed in PAIRS of tiles to amortize op overheads
    e_t = [a(f"e{i}", [P, 2 * K], fp32) for i in range(2)]
    ssum = [a(f"ssum{i}", [P, 2], fp32) for i in range(2)]
    recip = a("recip", [P, 2], fp32)
    en = a("en", [P, 2 * K], fp32)
    w2 = a("w2", [P, 2 * K], bf16)
    vtd = [a(f"vtd{i}", [P, 2 * K], bf16) for i in range(2)]
    bdw = [a(f"bdw{i}", [P, 2 * P], bf16) for i in range(2)]
    xt_s = [a(f"xt_s{i}", [D, P], fp32) for i in range(2)]
    yt_s = [a(f"yt_s{i}", [D, P], fp32) for i in range(2)]
    y_s = [a(f"y_s{i}", [P, D], fp32) for i in range(2)]
    wt = a("wt", [D, D], fp32)
    bias = a("bias", [D, 1], fp32)
    ident = a("ident", [P, P], fp32)
    # Column-permuted identity: perm[p, 4j+c] = 1 iff p == 32c+j.  Used as the
    # transpose rhs so selfT lands in (j,c)-ordered columns; each agg matmul
    # then accumulates into a CONTIGUOUS 4-column PSUM slice.  The row
    # permutation is undone on the host after the gather.
    perm = a("perm", [P, P], fp32)

    ap = nc.alloc_psum_tensor
    agg_ps = [ap(f"agg{i}", [D, P], fp32) for i in range(2)]
    yt_ps = [ap(f"yt{i}", [D, P], fp32) for i in range(2)]
    y_ps = [ap(f"y{i}", [P, D], fp32) for i in range(2)]

    s = nc.alloc_semaphore
    # DMA-completion sems are parity-split: two same-sem DMAs in flight can
    # interleave their 16 per-engine increments, so value 16 would not prove
    # the first transfer finished.  Parity buffers serialize same-sem use.
    s_rel = [s("s_rel0"), s("s_rel1"), s("s_rel2")]
    s_nei = [s(f"s_nei{i}") for i in range(6)]
    s_nep = [s(f"s_nep{i}") for i in range(6)]  # SWDGE-owned (POOL ring)
    s_out = [s("s_out0"), s("s_out1")]
    s_pre, s_init = s("s_pre"), s("s_init")
    s_scores, s_exp, s_bdw, s_xts = s("s_scores"), s("s_exp"), s("s_bdw"), s("s_xts")
    s_relu, s_ys = s("s_relu"), s("s_ys")
    s_agg, s_wt, s_ytr = s("s_agg"), s("s_wt"), s("s_ytr")
    s_dve = s("s_dve")  # same-engine DVE RAW ordering (deep pipeline)
    s_vt = s("s_vt")    # vtrans(u) completions

    def wge(eng, sem, val):
        if val > 0:
            eng.wait_ge(sem, val)

    TAIL = 10

    with nc.Block() as block:

        @block.gpsimd
        def _(g):
            # GpSimd's 8 Q7 cores run ops out of order; sem-gate the
            # affine_select behind the memset it reads.
            g.memset(bdw[0].ap(), 0.0).then_inc(s_init, 1)
            g.memset(bdw[1].ap(), 0.0).then_inc(s_init, 1)
            g.memset(ident.ap(), 0.0).then_inc(s_init, 1)
            g.memset(perm.ap(), 0.0).then_inc(s_init, 1)
            g.wait_ge(s_init, 4)
            g.affine_select(
                out=ident.ap(), in_=ident.ap(),
                compare_op=ALU.not_equal, fill=1.0, base=0,
                pattern=[[-1, P]], channel_multiplier=1,
            ).then_inc(s_init, 1)
            pv = perm.ap().rearrange("p (j c) -> p c j", c=4)
            for c in range(4):
                g.affine_select(
                    out=pv[:, c, :], in_=pv[:, c, :],
                    compare_op=ALU.not_equal, fill=1.0, base=-32 * c,
                    pattern=[[-1, 32]], channel_multiplier=1,
                ).then_inc(s_init, 1)
            for t in range(T + TAIL):
                if t < T:
                    wge(g, s_agg, t - 5)
                    for c in (2, 3):
                        g.dma_start(
                            nei[t % 6].ap()[32 * c:32 * c + 32, :].rearrange(
                                "p (j d) -> p j d", d=D),
                            nv_g[t, c],
                        ).then_inc(s_nep[t % 6], 16)
                w2 = t - 9
                if 0 <= w2 < T:
                    wge(g, s_ys, w2 + 1)
                    g.dma_start(out_d.ap()[w2 * P:w2 * P + P, :],
                                y_s[w2 % 2].ap()).then_inc(s_out[w2 % 2], 16)

        # Software-pipelined schedule.  In iteration t:
        #   SP : loads(t)
        #   DVE: TT(t), reduce(t), recip(t-1), en(t-1), TS(t-1), vtrans(t-1),
        #        xadd(t-3)
        #   ACT: copies(t-2), self_s(t-3), relu(t-5), y_s(t-6), out(t-6), exp(t)
        #   PE : selfT(t-2), aggs(t-2), WT(t-4), ytr(t-5)
        # Stage offsets are chosen so that EVERY cross-engine gate is
        # produced in a previous iteration -- each engine streams freely:
        #   SP  : loads(t)           POOL: nei c=2,3 (t), out(t-9)
        #   DVE : TT(t), reduce(t), softmax-tail(t-2) incl vtrans(t-2)
        #   ACT : copies(t-3), xt(t-5), relu(t-7), ys(t-9), exp(t)
        #   PE  : selfT+aggs(t-4), WT(t-6), ytr(t-8)

        @block.sync
        def _(sp):
            with nc.allow_non_contiguous_dma(reason="one-time preloads"):
                sp.dma_start(wt.ap(), w_d.ap().rearrange("j d -> d j")).then_inc(s_pre, 16)
                sp.dma_start(bias.ap(), b_d.ap()[:, None]).then_inc(s_pre, 16)
                sp.dma_start(self_all.ap().rearrange("p (t d) -> p t d", d=D),
                             self_cols).then_inc(s_pre, 16)
                sp.dma_start(norm_all.ap().rearrange("p (t k) -> p t k", k=K),
                             nn_cols).then_inc(s_pre, 16)
                sp.dma_start(user_all.ap().rearrange("p (m d) -> p m d", d=D),
                             user_cols[:, :UB, :]).then_inc(s_pre, 16)
            for t in range(T + TAIL):
                if t < T:
                    wge(sp, s_scores, t - 2)  # rel[t%3] free: TT(t-3) done
                    wge(sp, s_agg, t - 5)     # nei[t%6] free: aggs(t-6) done
                    sp.dma_start(rel[t % 3].ap(),
                                 rel_rows[t * P:t * P + P, :]).then_inc(s_rel[t % 3], 16)
                    for c in (0, 1):
                        sp.dma_start(
                            nei[t % 6].ap()[32 * c:32 * c + 32, :].rearrange(
                                "p (j d) -> p j d", d=D),
                            nv_g[t, c],
                        ).then_inc(s_nei[t % 6], 16)

        # s_dve counts are tracked explicitly because head/tail iterations
        # skip stages.
        cnt = {"n": 0}
        c_tt, c_en, c_ts = {}, {}, {}

        @block.vector
        def _(v):
            wge(v, s_init, 9)
            wge(v, s_pre, 80)
            for t in range(T + TAIL):
                if t < T:
                    i = t % 2
                    ut = user_all.ap()[:, D * (t % UB):D * (t % UB) + D]
                    wge(v, s_scores, t)       # prod WAR: reduce(t-1) done
                    wge(v, s_rel[t % 3], 16 * (t // 3 + 1))
                    wge(v, s_exp, t - 1)      # scores[i] WAR: exp(t-2) done
                    prod_v = prod.ap().rearrange("p (k d) -> p k d", k=K)
                    nc.vector.tensor_tensor(
                        prod_v, rel[t % 3].ap().rearrange("p (k d) -> p k d", k=K),
                        ut[:, None, :].to_broadcast((P, K, D)), ALU.mult
                    ).then_inc(s_dve, 1)
                    cnt["n"] += 1
                    c_tt[t] = cnt["n"]
                    wge(v, s_dve, c_tt[t])
                    nc.vector.reduce_sum(
                        scores[i].ap(), prod_v, axis=mybir.AxisListType.X
                    ).then_inc(s_scores, 1)
                u = t - 2
                if 0 <= u < T and u % 2 == 1:
                    pc = u // 2              # pair (2pc, 2pc+1)
                    nt = norm_all.ap()[:, 2 * K * pc:2 * K * pc + 2 * K]
                    wge(v, s_exp, u + 1)
                    nc.vector.reciprocal(recip.ap(), ssum[pc % 2].ap()).then_inc(s_dve, 1)
                    cnt["n"] += 1
                    nc.vector.tensor_tensor(
                        en.ap(), e_t[pc % 2].ap(), nt, ALU.mult
                    ).then_inc(s_dve, 1)
                    cnt["n"] += 1
                    c_en[pc] = cnt["n"]
                    wge(v, s_dve, c_en[pc])
                    wge(v, s_vt, pc)          # w2 WAR: vtrans(pc-1) done
                    nc.vector.tensor_tensor(
                        w2.ap().rearrange("p (b k) -> p b k", b=2),
                        en.ap().rearrange("p (b k) -> p b k", b=2),
                        recip.ap()[:, :, None].to_broadcast((P, 2, K)), ALU.mult
                    ).then_inc(s_dve, 1)
                    cnt["n"] += 1
                    c_ts[pc] = cnt["n"]
                    wge(v, s_dve, c_ts[pc])
                    wge(v, s_bdw, pc - 1)     # vtd[pc%2] WAR: copies(pc-2) done
                    nc.vector.transpose(vtd[pc % 2].ap(), w2.ap()).then_inc(s_vt, 1)

        @block.scalar
        def _(sc):
            wge(sc, s_pre, 80)
            for t in range(T + TAIL):
                u2 = t - 3
                if 0 <= u2 < T and u2 % 2 == 1:
                    pc2 = u2 // 2
                    wge(sc, s_vt, pc2 + 1)
                    wge(sc, s_agg, 2 * pc2 - 2)  # bdw[pc2%2] WAR: aggs(pair-2)
                    bvp = bdw[pc2 % 2].ap().rearrange(
                        "p (b j c) -> p c b j", b=2, c=4)
                    vv = vtd[pc2 % 2].ap().rearrange("p (b j) -> p b j", b=2)
                    for c in range(4):
                        inst = nc.scalar.copy(
                            out=bvp[32 * c:32 * c + 32, c],
                            in_=vv[32 * c:32 * c + 32])
                    inst.then_inc(s_bdw, 1)
                u3 = t - 5
                if 0 <= u3 < T:
                    wge(sc, s_agg, u3 + 1)    # selfT+aggs(u3) in PSUM
                    wge(sc, s_wt, u3 - 1)     # xt_s[u3%2] WAR: WT(u3-2) done
                    nc.scalar.copy(xt_s[u3 % 2].ap(),
                                   agg_ps[u3 % 2].ap()).then_inc(s_xts, 1)
                x2 = t - 7
                if 0 <= x2 < T:
                    wge(sc, s_ytr, x2 - 1)    # yt_s[x2%2] WAR: ytr(x2-2) done
                    wge(sc, s_wt, x2 + 1)
                    nc.scalar.activation(
                        yt_s[x2 % 2].ap(), yt_ps[x2 % 2].ap(), AF.Relu,
                        bias=bias.ap()
                    ).then_inc(s_relu, 1)
                w2 = t - 9
                if 0 <= w2 < T:
                    wge(sc, s_out[w2 % 2], 16 * (w2 // 2))  # y_s WAR: out(w2-2)
                    wge(sc, s_ytr, w2 + 1)
                    nc.scalar.copy(y_s[w2 % 2].ap(),
                                   y_ps[w2 % 2].ap()).then_inc(s_ys, 1)
                if t < T:
                    if (t // 2 - 2) in c_en:
                        wge(sc, s_dve, c_en[t // 2 - 2])  # e/ssum buffer WAR
                    wge(sc, s_scores, t + 1)
                    h = t % 2
                    nc.scalar.activation(
                        e_t[(t // 2) % 2].ap()[:, K * h:K * h + K],
                        scores[t % 2].ap(), AF.Exp,
                        accum_out=ssum[(t // 2) % 2].ap()[:, h:h + 1]
                    ).then_inc(s_exp, 1)

        @block.tensor
        def _(pe):
            wge(pe, s_init, 9)
            wge(pe, s_pre, 80)
            for t in range(T + TAIL):
                u = t - 4
                if 0 <= u < T:
                    st = self_all.ap()[:, D * u:D * u + D]
                    wge(pe, s_xts, u - 1)     # agg_ps[u%2] WAR: xt_s(u-2) done
                    nc.tensor.matmul(agg_ps[u % 2].ap(), st, perm.ap(),
                                     is_transpose=True, start=True, stop=False)
                    wge(pe, s_bdw, u // 2 + 1)
                    wge(pe, s_nei[u % 6], 32 * (u // 6 + 1))
                    wge(pe, s_nep[u % 6], 32 * (u // 6 + 1))
                    boff = P * (u % 2)
                    for j in range(K):
                        inst = nc.tensor.matmul(
                            agg_ps[u % 2].ap()[:, 4 * j:4 * j + 4],
                            nei[u % 6].ap()[:, D * j:D * j + D],
                            bdw[(u // 2) % 2].ap()[:, boff + 4 * j:boff + 4 * j + 4],
                            start=False, stop=(j == K - 1))
                    inst.then_inc(s_agg, 1)
                x3 = t - 6
                if 0 <= x3 < T:
                    wge(pe, s_xts, x3 + 1)
                    wge(pe, s_relu, x3 - 1)   # yt_ps[x3%2] WAR: relu(x3-2) done
                    nc.tensor.matmul(yt_ps[x3 % 2].ap(), wt.ap(),
                                     xt_s[x3 % 2].ap(),
                                     start=True, stop=True).then_inc(s_wt, 1)
                w = t - 8
                if 0 <= w < T:
                    wge(pe, s_relu, w + 1)
                    wge(pe, s_ys, w - 1)      # y_ps[w%2] WAR: y_s(w-2) done
                    nc.tensor.matmul(y_ps[w % 2].ap(), yt_s[w % 2].ap(),
                                     ident.ap()[:D, :D],
                                     is_transpose=True, start=True, stop=True
                                     ).then_inc(s_ytr, 1)

    return nc


# PE writes each 128-row tile with rows in (j,c) order: output row 4*j+c
# holds logical row 32*c+j.  Undo after the gather.
_ROWPERM = np.arange(P).reshape(32, 4).T.reshape(P)  # q(lr) = 4*(lr%32)+lr//32


def unpermute(out_rows):
    r = out_rows.reshape(-1, P, D)
    return r[:, _ROWPERM, :].reshape(out_rows.shape)


_NC_CACHE: dict = {}


def _get_nc(r_loc: int):
    if r_loc not in _NC_CACHE:
        _NC_CACHE[r_loc] = build_nc(r_loc)
    return _NC_CACHE[r_loc]


def kernel(self_vectors, neighbor_vectors, neighbor_relations, neighbor_norms,
           user_embeddings, W, b, trace: bool = False):
    import ml_dtypes

    from concourse.bass_utils import run_bass_kernel_spmd

    nc = _get_nc(R_LOC)
    nloc = R_LOC * K
    # bf16 inputs for the scores TT (DVE 2x mode) and the aggregation
    # matmuls (single-pass PE); halves their HBM traffic.  Verified
    # end-to-end rel_l2 impact: 1.1e-04.
    nv_bf = np.ascontiguousarray(neighbor_vectors).astype(ml_dtypes.bfloat16)
    nr_bf = np.ascontiguousarray(neighbor_relations).astype(ml_dtypes.bfloat16)
    user_bf = np.ascontiguousarray(user_embeddings).astype(ml_dtypes.bfloat16)
    nnk = (np.ascontiguousarray(neighbor_norms) * np.float32(1.0 / K)).astype(np.float32)
    in_maps = []
    for s in range(NCORES):
        r0 = s * R_LOC
        n0 = r0 * K
        in_maps.append({
            "self_v": np.ascontiguousarray(self_vectors[r0:r0 + R_LOC]),
            "nei_v": nv_bf[n0:n0 + nloc],
            "nei_r": nr_bf[n0:n0 + nloc],
            "nei_n": nnk[n0:n0 + nloc],
            "user": user_bf,
            "W": np.ascontiguousarray(W),
            "b": np.ascontiguousarray(b),
        })
    res = run_bass_kernel_spmd(nc, in_maps, core_ids=list(range(NCORES)),
                               trace=trace)
    out = np.concatenate([unpermute(res.results[s]["out"])
                          for s in range(NCORES)], axis=0)
    if trace:
        return out, res
    return out


# revision 38
# speedup vs baseline: 1.0163x; 1.0163x over previous
"""Trainium2 Bass kernel for the GNN neighbor-aggregation module.

Computation (per row r with K=32 neighbors, D=64):
    scores[r,k]  = sum_d rel[r,k,d] * user[r%B, d]
    w[r,:]       = softmax(scores[r,:])                (no max-subtract; |score|<60)
    agg[r,d]     = (1/K) sum_k nei[r,k,d] * w[r,k] * norms[r,k]
    out[r,:]     = relu((self[r,:] + agg[r,:]) @ W.T + b)

Sharding: pure data parallelism over rows across 8 cores (8192 rows/core).
W, b and user_embeddings are replicated (user index = r mod 4096, and
8192 % 4096 == 0, so local user indexing is identical on every shard).

Raw-bass implementation (this walrus build rejects instructions carrying
embedded multi-sem waits, which rules out TileContext): explicit semaphores,
standalone wait_ge instructions, double-buffered SBUF/PSUM, per-engine
programs via nc.Block().

Precision: neighbor_relations / neighbor_vectors / user_embeddings are cast
to bf16 on the host (halves their HBM traffic, enables the DVE 2x tensor-
tensor mode and single-pass PE matmuls).  Everything downstream of the
softmax numerator stays fp32.  Verified end-to-end rel_l2 vs the fp32
reference: 1.1e-04.

Per 128-row tile:
  DVE : scores = segmented-reduce(rel * user_bcast); softmax small ops;
        32x32 block-transpose of the weights; scatter into a block-diagonal
        [128,128] bf16 tile; x^T = agg^T + self^T.
  ACT : exp (with fused row-sum accumulator), PSUM->SBUF copies, relu+bias.
  PE  : 32 block-diagonal bf16 matmuls (lhsT = nei tile [128=(c,k), 64d],
        rhs = 4 block-diag weight columns) producing agg^T[d, r] directly
        in PSUM; self^T via PE transpose; y^T = W^T.T @ x^T; transpose back.
  DMA : self/norms/user fully preloaded at startup; per tile only
        rel (SP ring), 4 nei c-block transfers split SP/POOL-SWDGE rings,
        and the output store (SP).
"""

import numpy as np

import concourse.bass as bass
import concourse.mybir as mybir

fp32 = mybir.dt.float32
bf16 = mybir.dt.bfloat16
AF = mybir.ActivationFunctionType
ALU = mybir.AluOpType

R, D, B, K = 65536, 64, 4096, 32
NCORES = 8
R_LOC = R // NCORES  # 8192 rows per core
P = 128              # rows per tile


def build_nc(r_loc: int = R_LOC):
    nc = bass.Bass("TRN2", target_bir_lowering=False, debug=False)

    self_d = nc.dram_tensor("self_v", [r_loc, D], fp32, kind="ExternalInput")
    nv_d = nc.dram_tensor("nei_v", [r_loc * K, D], bf16, kind="ExternalInput")
    nr_d = nc.dram_tensor("nei_r", [r_loc * K, D], bf16, kind="ExternalInput")
    nn_d = nc.dram_tensor("nei_n", [r_loc * K], fp32, kind="ExternalInput")
    user_d = nc.dram_tensor("user", [B, D], bf16, kind="ExternalInput")
    w_d = nc.dram_tensor("W", [D, D], fp32, kind="ExternalInput")
    b_d = nc.dram_tensor("b", [D], fp32, kind="ExternalInput")
    out_d = nc.dram_tensor("out", [r_loc, D], fp32, kind="ExternalOutput")

    T = r_loc // P       # number of 128-row tiles
    UB = min(B, r_loc) // P  # distinct 128-row user blocks

    # DRAM views
    rel_rows = nr_d.ap().rearrange("(r k) d -> r (k d)", k=K)    # [r_loc, 2048]
    # nei matmul layout: nei[32c+k, 64j+d] = nv[(t*128+32c+j)*K + k, d],
    # one DMA per 32-partition block c with an affine [k, j, d] AP.
    nv_g = nv_d.ap().rearrange("(t c j k) d -> t c k j d", c=4, j=32, k=K)
    # preload views: all tiles side by side in the free dimension
    self_cols = self_d.ap().rearrange("(t p) d -> p t d", p=P)   # [128, T, 64]
    nn_cols = nn_d.ap().rearrange("(t p k) -> p t k", p=P, k=K)  # [128, T, 32]
    user_cols = user_d.ap().rearrange("(m p) d -> p m d", p=P)   # [128, B/128, 64]

    a = nc.alloc_sbuf_tensor
    rel = [a(f"rel{i}", [P, K * D], bf16) for i in range(3)]
    nei = [a(f"nei{i}", [P, K * D], bf16) for i in range(6)]
    prod = a("prod", [P, K * D], bf16)
    self_all = a("self_all", [P, T * D], fp32)
    norm_all = a("norm_all", [P, T * K], fp32)
    user_all = a("user_all", [P, UB * D], bf16)
    scores = [a(f"scores{i}", [P, K], fp32) for i in range(2)]
    # softmax tail is processed in PAIRS of tiles to amortize op overheads
    e_t = [a(f"e{i}", [P, 2 * K], fp32) for i in range(2)]
    ssum = [a(f"ssum{i}", [P, 2], fp32) for i in range(2)]
    recip = a("recip", [P, 2], fp32)
    en = a("en", [P, 2 * K], fp32)
    w2 = a("w2", [P, 2 * K], bf16)
    vtd = [a(f"vtd{i}", [P, 2 * K], bf16) for i in range(2)]
    bdw = [a(f"bdw{i}", [P, 2 * P], bf16) for i in range(2)]
    xt_s = [a(f"xt_s{i}", [D, P], fp32) for i in range(2)]
    yt_s = [a(f"yt_s{i}", [D, P], fp32) for i in range(2)]
    y_s = [a(f"y_s{i}", [P, D], fp32) for i in range(2)]
    wt = a("wt", [D, D], fp32)
    bias = a("bias", [D, 1], fp32)
    ident = a("ident", [P, P], fp32)
    # Column-permuted identity: perm[p, 4j+c] = 1 iff p == 32c+j.  Used as the
    # transpose rhs so selfT lands in (j,c)-ordered columns; each agg matmul
    # then accumulates into a CONTIGUOUS 4-column PSUM slice.  The row
    # permutation is undone on the host after the gather.
    perm = a("perm", [P, P], fp32)

    ap = nc.alloc_psum_tensor
    agg_ps = [ap(f"agg{i}", [D, P], fp32) for i in range(2)]
    yt_ps = [ap(f"yt{i}", [D, P], fp32) for i in range(2)]
    y_ps = [ap(f"y{i}", [P, D], fp32) for i in range(2)]

    s = nc.alloc_semaphore
    # DMA-completion sems are parity-split: two same-sem DMAs in flight can
    # interleave their 16 per-engine increments, so value 16 would not prove
    # the first transfer finished.  Parity buffers serialize same-sem use.
    s_rel = [s("s_rel0"), s("s_rel1"), s("s_rel2")]
    s_nei = [s(f"s_nei{i}") for i in range(6)]
    s_nep = [s(f"s_nep{i}") for i in range(6)]  # SWDGE-owned (POOL ring)
    s_out = [s("s_out0"), s("s_out1")]
    s_pre, s_init = s("s_pre"), s("s_init")
    s_scores, s_exp, s_bdw, s_xts = s("s_scores"), s("s_exp"), s("s_bdw"), s("s_xts")
    s_relu, s_ys = s("s_relu"), s("s_ys")
    s_agg, s_wt, s_ytr = s("s_agg"), s("s_wt"), s("s_ytr")
    s_dve = s("s_dve")  # same-engine DVE RAW ordering (deep pipeline)
    s_vt = s("s_vt")    # vtrans(u) completions

    def wge(eng, sem, val):
        if val > 0:
            eng.wait_ge(sem, val)

    TAIL = 10

    with nc.Block() as block:

        @block.gpsimd
        def _(g):
            # GpSimd's 8 Q7 cores run ops out of order; sem-gate the
            # affine_select behind the memset it reads.
            g.memset(bdw[0].ap(), 0.0).then_inc(s_init, 1)
            g.memset(bdw[1].ap(), 0.0).then_inc(s_init, 1)
            g.memset(ident.ap(), 0.0).then_inc(s_init, 1)
            g.memset(perm.ap(), 0.0).then_inc(s_init, 1)
            g.wait_ge(s_init, 4)
            g.affine_select(
                out=ident.ap(), in_=ident.ap(),
                compare_op=ALU.not_equal, fill=1.0, base=0,
                pattern=[[-1, P]], channel_multiplier=1,
            ).then_inc(s_init, 1)
            pv = perm.ap().rearrange("p (j c) -> p c j", c=4)
            for c in range(4):
                g.affine_select(
                    out=pv[:, c, :], in_=pv[:, c, :],
                    compare_op=ALU.not_equal, fill=1.0, base=-32 * c,
                    pattern=[[-1, 32]], channel_multiplier=1,
                ).then_inc(s_init, 1)
            for t in range(T + TAIL):
                if t < T:
                    wge(g, s_agg, t - 5)
                    for c in (2, 3):
                        g.dma_start(
                            nei[t % 6].ap()[32 * c:32 * c + 32, :].rearrange(
                                "p (j d) -> p j d", d=D),
                            nv_g[t, c],
                        ).then_inc(s_nep[t % 6], 16)
                w2 = t - 9
                if 0 <= w2 < T:
                    wge(g, s_ys, w2 + 1)
                    g.dma_start(out_d.ap()[w2 * P:w2 * P + P, :],
                                y_s[w2 % 2].ap()).then_inc(s_out[w2 % 2], 16)

        # Software-pipelined schedule.  In iteration t:
        #   SP : loads(t)
        #   DVE: TT(t), reduce(t), recip(t-1), en(t-1), TS(t-1), vtrans(t-1),
        #        xadd(t-3)
        #   ACT: copies(t-2), self_s(t-3), relu(t-5), y_s(t-6), out(t-6), exp(t)
        #   PE : selfT(t-2), aggs(t-2), WT(t-4), ytr(t-5)
        # Stage offsets are chosen so that EVERY cross-engine gate is
        # produced in a previous iteration -- each engine streams freely:
        #   SP  : loads(t)           POOL: nei c=2,3 (t), out(t-9)
        #   DVE : TT(t), reduce(t), softmax-tail(t-2) incl vtrans(t-2)
        #   ACT : copies(t-3), xt(t-5), relu(t-7), ys(t-9), exp(t)
        #   PE  : selfT+aggs(t-4), WT(t-6), ytr(t-8)

        @block.sync
        def _(sp):
            with nc.allow_non_contiguous_dma(reason="one-time preloads"):
                sp.dma_start(wt.ap(), w_d.ap().rearrange("j d -> d j")).then_inc(s_pre, 16)
                sp.dma_start(bias.ap(), b_d.ap()[:, None]).then_inc(s_pre, 16)
                sp.dma_start(self_all.ap().rearrange("p (t d) -> p t d", d=D),
                             self_cols).then_inc(s_pre, 16)
                sp.dma_start(norm_all.ap().rearrange("p (t k) -> p t k", k=K),
                             nn_cols).then_inc(s_pre, 16)
                sp.dma_start(user_all.ap().rearrange("p (m d) -> p m d", d=D),
                             user_cols[:, :UB, :]).then_inc(s_pre, 16)
            for t in range(T + TAIL):
                if t < T:
                    wge(sp, s_scores, t - 2)  # rel[t%3] free: TT(t-3) done
                    wge(sp, s_agg, t - 5)     # nei[t%6] free: aggs(t-6) done
                    sp.dma_start(rel[t % 3].ap(),
                                 rel_rows[t * P:t * P + P, :]).then_inc(s_rel[t % 3], 16)
                    for c in (0, 1):
                        sp.dma_start(
                            nei[t % 6].ap()[32 * c:32 * c + 32, :].rearrange(
                                "p (j d) -> p j d", d=D),
                            nv_g[t, c],
                        ).then_inc(s_nei[t % 6], 16)

        # s_dve counts are tracked explicitly because head/tail iterations
        # skip stages.
        cnt = {"n": 0}
        c_tt, c_en, c_ts = {}, {}, {}

        @block.vector
        def _(v):
            wge(v, s_init, 9)
            wge(v, s_pre, 80)
            for t in range(T + TAIL):
                if t < T:
                    i = t % 2
                    ut = user_all.ap()[:, D * (t % UB):D * (t % UB) + D]
                    wge(v, s_scores, t)       # prod WAR: reduce(t-1) done
                    wge(v, s_rel[t % 3], 16 * (t // 3 + 1))
                    wge(v, s_exp, t - 1)      # scores[i] WAR: exp(t-2) done
                    prod_v = prod.ap().rearrange("p (k d) -> p k d", k=K)
                    nc.vector.tensor_tensor(
                        prod_v, rel[t % 3].ap().rearrange("p (k d) -> p k d", k=K),
                        ut[:, None, :].to_broadcast((P, K, D)), ALU.mult
                    ).then_inc(s_dve, 1)
                    cnt["n"] += 1
                    c_tt[t] = cnt["n"]
                    wge(v, s_dve, c_tt[t])
                    nc.vector.reduce_sum(
                        scores[i].ap(), prod_v, axis=mybir.AxisListType.X
                    ).then_inc(s_scores, 1)
                u = t - 2
                if 0 <= u < T and u % 2 == 1:
                    pc = u // 2              # pair (2pc, 2pc+1)
                    nt = norm_all.ap()[:, 2 * K * pc:2 * K * pc + 2 * K]
                    wge(v, s_exp, u + 1)
                    nc.vector.reciprocal(recip.ap(), ssum[pc % 2].ap()).then_inc(s_dve, 1)
                    cnt["n"] += 1
                    nc.vector.tensor_tensor(
                        en.ap(), e_t[pc % 2].ap(), nt, ALU.mult
                    ).then_inc(s_dve, 1)
                    cnt["n"] += 1
                    c_en[pc] = cnt["n"]
                    wge(v, s_dve, c_en[pc])
                    wge(v, s_vt, pc)          # w2 WAR: vtrans(pc-1) done
                    nc.vector.tensor_tensor(
                        w2.ap().rearrange("p (b k) -> p b k", b=2),
                        en.ap().rearrange("p (b k) -> p b k", b=2),
                        recip.ap()[:, :, None].to_broadcast((P, 2, K)), ALU.mult
                    ).then_inc(s_dve, 1)
                    cnt["n"] += 1
                    c_ts[pc] = cnt["n"]
                    wge(v, s_dve, c_ts[pc])
                    wge(v, s_bdw, pc - 1)     # vtd[pc%2] WAR: copies(pc-2) done
                    nc.vector.transpose(vtd[pc % 2].ap(), w2.ap()).then_inc(s_vt, 1)

        @block.scalar
        def _(sc):
            wge(sc, s_pre, 80)
            for t in range(T + TAIL):
                te = t - 1
                if 0 <= te < T:
                    if (te // 2 - 2) in c_en:
                        wge(sc, s_dve, c_en[te // 2 - 2])  # e/ssum buffer WAR
                    wge(sc, s_scores, te + 1)
                    h = te % 2
                    nc.scalar.activation(
                        e_t[(te // 2) % 2].ap()[:, K * h:K * h + K],
                        scores[te % 2].ap(), AF.Exp,
                        accum_out=ssum[(te // 2) % 2].ap()[:, h:h + 1]
                    ).then_inc(s_exp, 1)
                u2 = t - 3
                if 0 <= u2 < T and u2 % 2 == 1:
                    pc2 = u2 // 2
                    wge(sc, s_vt, pc2 + 1)
                    wge(sc, s_agg, 2 * pc2 - 2)  # bdw[pc2%2] WAR: aggs(pair-2)
                    bvp = bdw[pc2 % 2].ap().rearrange(
                        "p (b j c) -> p c b j", b=2, c=4)
                    vv = vtd[pc2 % 2].ap().rearrange("p (b j) -> p b j", b=2)
                    for c in range(4):
                        inst = nc.scalar.copy(
                            out=bvp[32 * c:32 * c + 32, c],
                            in_=vv[32 * c:32 * c + 32])
                    inst.then_inc(s_bdw, 1)
                u3 = t - 5
                if 0 <= u3 < T:
                    wge(sc, s_agg, u3 + 1)    # selfT+aggs(u3) in PSUM
                    wge(sc, s_wt, u3 - 1)     # xt_s[u3%2] WAR: WT(u3-2) done
                    nc.scalar.copy(xt_s[u3 % 2].ap(),
                                   agg_ps[u3 % 2].ap()).then_inc(s_xts, 1)
                x2 = t - 7
                if 0 <= x2 < T:
                    wge(sc, s_ytr, x2 - 1)    # yt_s[x2%2] WAR: ytr(x2-2) done
                    wge(sc, s_wt, x2 + 1)
                    nc.scalar.activation(
                        yt_s[x2 % 2].ap(), yt_ps[x2 % 2].ap(), AF.Relu,
                        bias=bias.ap()
                    ).then_inc(s_relu, 1)
                w2 = t - 9
                if 0 <= w2 < T:
                    wge(sc, s_out[w2 % 2], 16 * (w2 // 2))  # y_s WAR: out(w2-2)
                    wge(sc, s_ytr, w2 + 1)
                    nc.scalar.copy(y_s[w2 % 2].ap(),
                                   y_ps[w2 % 2].ap()).then_inc(s_ys, 1)


        @block.tensor
        def _(pe):
            wge(pe, s_init, 9)
            wge(pe, s_pre, 80)
            for t in range(T + TAIL):
                u = t - 4
                if 0 <= u < T:
                    st = self_all.ap()[:, D * u:D * u + D]
                    wge(pe, s_xts, u - 1)     # agg_ps[u%2] WAR: xt_s(u-2) done
                    nc.tensor.matmul(agg_ps[u % 2].ap(), st, perm.ap(),
                                     is_transpose=True, start=True, stop=False)
                    wge(pe, s_bdw, u // 2 + 1)
                    wge(pe, s_nei[u % 6], 32 * (u // 6 + 1))
                    wge(pe, s_nep[u % 6], 32 * (u // 6 + 1))
                    boff = P * (u % 2)
                    for j in range(K):
                        inst = nc.tensor.matmul(
                            agg_ps[u % 2].ap()[:, 4 * j:4 * j + 4],
                            nei[u % 6].ap()[:, D * j:D * j + D],
                            bdw[(u // 2) % 2].ap()[:, boff + 4 * j:boff + 4 * j + 4],
                            start=False, stop=(j == K - 1))
                    inst.then_inc(s_agg, 1)
                x3 = t - 6
                if 0 <= x3 < T:
                    wge(pe, s_xts, x3 + 1)
                    wge(pe, s_relu, x3 - 1)   # yt_ps[x3%2] WAR: relu(x3-2) done
                    nc.tensor.matmul(yt_ps[x3 % 2].ap(), wt.ap(),
                                     xt_s[x3 % 2].ap(),
                                     start=True, stop=True).then_inc(s_wt, 1)
                w = t - 8
                if 0 <= w < T:
                    wge(pe, s_relu, w + 1)
                    wge(pe, s_ys, w - 1)      # y_ps[w%2] WAR: y_s(w-2) done
                    nc.tensor.matmul(y_ps[w % 2].ap(), yt_s[w % 2].ap(),
                                     ident.ap()[:D, :D],
                                     is_transpose=True, start=True, stop=True
                                     ).then_inc(s_ytr, 1)

    return nc


# PE writes each 128-row tile with rows in (j,c) order: output row 4*j+c
# holds logical row 32*c+j.  Undo after the gather.
_ROWPERM = np.arange(P).reshape(32, 4).T.reshape(P)  # q(lr) = 4*(lr%32)+lr//32


def unpermute(out_rows):
    r = out_rows.reshape(-1, P, D)
    return r[:, _ROWPERM, :].reshape(out_rows.shape)


_NC_CACHE: dict = {}


def _get_nc(r_loc: int):
    if r_loc not in _NC_CACHE:
        _NC_CACHE[r_loc] = build_nc(r_loc)
    return _NC_CACHE[r_loc]


def kernel(self_vectors, neighbor_vectors, neighbor_relations, neighbor_norms,
           user_embeddings, W, b, trace: bool = False):
    import ml_dtypes

    from concourse.bass_utils import run_bass_kernel_spmd

    nc = _get_nc(R_LOC)
    nloc = R_LOC * K
    # bf16 inputs for the scores TT (DVE 2x mode) and the aggregation
    # matmuls (single-pass PE); halves their HBM traffic.  Verified
    # end-to-end rel_l2 impact: 1.1e-04.
    nv_bf = np.ascontiguousarray(neighbor_vectors).astype(ml_dtypes.bfloat16)
    nr_bf = np.ascontiguousarray(neighbor_relations).astype(ml_dtypes.bfloat16)
    user_bf = np.ascontiguousarray(user_embeddings).astype(ml_dtypes.bfloat16)
    nnk = (np.ascontiguousarray(neighbor_norms) * np.float32(1.0 / K)).astype(np.float32)
    in_maps = []
    for s in range(NCORES):
        r0 = s * R_LOC
        n0 = r0 * K
        in_maps.append({
            "self_v": np.ascontiguousarray(self_vectors[r0:r0 + R_LOC]),
            "nei_v": nv_bf[n0:n0 + nloc],
            "nei_r": nr_bf[n0:n0 + nloc],
            "nei_n": nnk[n0:n0 + nloc],
            "user": user_bf,
            "W": np.ascontiguousarray(W),
            "b": np.ascontiguousarray(b),
        })
    res = run_bass_kernel_spmd(nc, in_maps, core_ids=list(range(NCORES)),
                               trace=trace)
    out = np.concatenate([unpermute(res.results[s]["out"])
                          for s in range(NCORES)], axis=0)
    if trace:
        return out, res
    return out


# revision 40
# speedup vs baseline: 1.0236x; 1.0072x over previous
"""Trainium2 Bass kernel for the GNN neighbor-aggregation module.

Computation (per row r with K=32 neighbors, D=64):
    scores[r,k]  = sum_d rel[r,k,d] * user[r%B, d]
    w[r,:]       = softmax(scores[r,:])                (no max-subtract; |score|<60)
    agg[r,d]     = (1/K) sum_k nei[r,k,d] * w[r,k] * norms[r,k]
    out[r,:]     = relu((self[r,:] + agg[r,:]) @ W.T + b)

Sharding: pure data parallelism over rows across 8 cores (8192 rows/core).
W, b and user_embeddings are replicated (user index = r mod 4096, and
8192 % 4096 == 0, so local user indexing is identical on every shard).

Raw-bass implementation (this walrus build rejects instructions carrying
embedded multi-sem waits, which rules out TileContext): explicit semaphores,
standalone wait_ge instructions, double-buffered SBUF/PSUM, per-engine
programs via nc.Block().

Precision: neighbor_relations / neighbor_vectors / user_embeddings are cast
to bf16 on the host (halves their HBM traffic, enables the DVE 2x tensor-
tensor mode and single-pass PE matmuls).  Everything downstream of the
softmax numerator stays fp32.  Verified end-to-end rel_l2 vs the fp32
reference: 1.1e-04.

Per 128-row tile:
  DVE : scores = segmented-reduce(rel * user_bcast); softmax small ops;
        32x32 block-transpose of the weights; scatter into a block-diagonal
        [128,128] bf16 tile; x^T = agg^T + self^T.
  ACT : exp (with fused row-sum accumulator), PSUM->SBUF copies, relu+bias.
  PE  : 32 block-diagonal bf16 matmuls (lhsT = nei tile [128=(c,k), 64d],
        rhs = 4 block-diag weight columns) producing agg^T[d, r] directly
        in PSUM; self^T via PE transpose; y^T = W^T.T @ x^T; transpose back.
  DMA : self/norms/user fully preloaded at startup; per tile only
        rel (SP ring), 4 nei c-block transfers split SP/POOL-SWDGE rings,
        and the output store (SP).
"""

import numpy as np

import concourse.bass as bass
import concourse.mybir as mybir

fp32 = mybir.dt.float32
bf16 = mybir.dt.bfloat16
AF = mybir.ActivationFunctionType
ALU = mybir.AluOpType

R, D, B, K = 65536, 64, 4096, 32
NCORES = 8
R_LOC = R // NCORES  # 8192 rows per core
P = 128              # rows per tile


def build_nc(r_loc: int = R_LOC):
    nc = bass.Bass("TRN2", target_bir_lowering=False, debug=False)

    self_d = nc.dram_tensor("self_v", [r_loc, D], fp32, kind="ExternalInput")
    nv_d = nc.dram_tensor("nei_v", [r_loc * K, D], bf16, kind="ExternalInput")
    nr_d = nc.dram_tensor("nei_r", [r_loc * K, D], bf16, kind="ExternalInput")
    nn_d = nc.dram_tensor("nei_n", [r_loc * K], fp32, kind="ExternalInput")
    user_d = nc.dram_tensor("user", [B, D], bf16, kind="ExternalInput")
    w_d = nc.dram_tensor("W", [D, D], fp32, kind="ExternalInput")
    b_d = nc.dram_tensor("b", [D], fp32, kind="ExternalInput")
    out_d = nc.dram_tensor("out", [r_loc, D], fp32, kind="ExternalOutput")

    T = r_loc // P       # number of 128-row tiles
    UB = min(B, r_loc) // P  # distinct 128-row user blocks

    # DRAM views
    rel_rows = nr_d.ap().rearrange("(r k) d -> r (k d)", k=K)    # [r_loc, 2048]
    # nei matmul layout: nei[32c+k, 64j+d] = nv[(t*128+32c+j)*K + k, d],
    # one DMA per 32-partition block c with an affine [k, j, d] AP.
    nv_g = nv_d.ap().rearrange("(t c j k) d -> t c k j d", c=4, j=32, k=K)
    # preload views: all tiles side by side in the free dimension
    self_cols = self_d.ap().rearrange("(t p) d -> p t d", p=P)   # [128, T, 64]
    nn_cols = nn_d.ap().rearrange("(t p k) -> p t k", p=P, k=K)  # [128, T, 32]
    user_cols = user_d.ap().rearrange("(m p) d -> p m d", p=P)   # [128, B/128, 64]

    a = nc.alloc_sbuf_tensor
    rel = [a(f"rel{i}", [P, K * D], bf16) for i in range(3)]
    nei = [a(f"nei{i}", [P, K * D], bf16) for i in range(6)]
    prod = a("prod", [P, K * D], bf16)
    self_all = a("self_all", [P, T * D], fp32)
    norm_all = a("norm_all", [P, T * K], fp32)
    user_all = a("user_all", [P, UB * D], bf16)
    scores = [a(f"scores{i}", [P, K], fp32) for i in range(2)]
    # softmax tail is processed in PAIRS of tiles to amortize op overheads
    e_t = [a(f"e{i}", [P, 2 * K], fp32) for i in range(2)]
    ssum = [a(f"ssum{i}", [P, 2], fp32) for i in range(2)]
    recip = a("recip", [P, 2], fp32)
    en = a("en", [P, 2 * K], fp32)
    w2 = a("w2", [P, 2 * K], bf16)
    vtd = [a(f"vtd{i}", [P, 2 * K], bf16) for i in range(2)]
    bdw = [a(f"bdw{i}", [P, 2 * P], bf16) for i in range(2)]
    xt_s = [a(f"xt_s{i}", [D, P], fp32) for i in range(2)]
    yt_s = [a(f"yt_s{i}", [D, P], fp32) for i in range(2)]
    y_s = [a(f"y_s{i}", [P, D], fp32) for i in range(2)]
    wt = a("wt", [D, D], fp32)
    bias = a("bias", [D, 1], fp32)
    ident = a("ident", [P, P], fp32)
    # Column-permuted identity: perm[p, 4j+c] = 1 iff p == 32c+j.  Used as the
    # transpose rhs so selfT lands in (j,c)-ordered columns; each agg matmul
    # then accumulates into a CONTIGUOUS 4-column PSUM slice.  The row
    # permutation is undone on the host after the gather.
    perm = a("perm", [P, P], fp32)

    ap = nc.alloc_psum_tensor
    agg_ps = [ap(f"agg{i}", [D, P], fp32) for i in range(2)]
    yt_ps = [ap(f"yt{i}", [D, P], fp32) for i in range(2)]
    y_ps = [ap(f"y{i}", [P, D], fp32) for i in range(2)]

    s = nc.alloc_semaphore
    # DMA-completion sems are parity-split: two same-sem DMAs in flight can
    # interleave their 16 per-engine increments, so value 16 would not prove
    # the first transfer finished.  Parity buffers serialize same-sem use.
    s_rel = [s("s_rel0"), s("s_rel1"), s("s_rel2")]
    s_nei = [s(f"s_nei{i}") for i in range(6)]
    s_nep = [s(f"s_nep{i}") for i in range(6)]  # SWDGE-owned (POOL ring)
    s_out = [s("s_out0"), s("s_out1")]
    s_pre, s_init = s("s_pre"), s("s_init")
    s_prep, s_prea = s("s_prep"), s("s_prea")  # POOL/ACT preloads
    s_scores, s_exp, s_bdw, s_xts = s("s_scores"), s("s_exp"), s("s_bdw"), s("s_xts")
    s_relu, s_ys = s("s_relu"), s("s_ys")
    s_agg, s_wt, s_ytr = s("s_agg"), s("s_wt"), s("s_ytr")
    s_dve = s("s_dve")  # same-engine DVE RAW ordering (deep pipeline)
    s_vt = s("s_vt")    # vtrans(u) completions

    def wge(eng, sem, val):
        if val > 0:
            eng.wait_ge(sem, val)

    TAIL = 10

    with nc.Block() as block:

        @block.gpsimd
        def _(g):
            # GpSimd's 8 Q7 cores run ops out of order; sem-gate the
            # affine_select behind the memset it reads.
            g.memset(bdw[0].ap(), 0.0).then_inc(s_init, 1)
            g.memset(bdw[1].ap(), 0.0).then_inc(s_init, 1)
            g.memset(ident.ap(), 0.0).then_inc(s_init, 1)
            g.memset(perm.ap(), 0.0).then_inc(s_init, 1)
            g.wait_ge(s_init, 4)
            g.affine_select(
                out=ident.ap(), in_=ident.ap(),
                compare_op=ALU.not_equal, fill=1.0, base=0,
                pattern=[[-1, P]], channel_multiplier=1,
            ).then_inc(s_init, 1)
            pv = perm.ap().rearrange("p (j c) -> p c j", c=4)
            for c in range(4):
                g.affine_select(
                    out=pv[:, c, :], in_=pv[:, c, :],
                    compare_op=ALU.not_equal, fill=1.0, base=-32 * c,
                    pattern=[[-1, 32]], channel_multiplier=1,
                ).then_inc(s_init, 1)
            with nc.allow_non_contiguous_dma(reason="one-time norm preload"):
                g.dma_start(norm_all.ap().rearrange("p (t k) -> p t k", k=K),
                            nn_cols).then_inc(s_prep, 16)
            for t in range(T + TAIL):
                if t < T:
                    wge(g, s_agg, t - 5)
                    for c in (2, 3):
                        g.dma_start(
                            nei[t % 6].ap()[32 * c:32 * c + 32, :].rearrange(
                                "p (j d) -> p j d", d=D),
                            nv_g[t, c],
                        ).then_inc(s_nep[t % 6], 16)
                w2 = t - 9
                if 0 <= w2 < T:
                    wge(g, s_ys, w2 + 1)
                    g.dma_start(out_d.ap()[w2 * P:w2 * P + P, :],
                                y_s[w2 % 2].ap()).then_inc(s_out[w2 % 2], 16)

        # Software-pipelined schedule.  In iteration t:
        #   SP : loads(t)
        #   DVE: TT(t), reduce(t), recip(t-1), en(t-1), TS(t-1), vtrans(t-1),
        #        xadd(t-3)
        #   ACT: copies(t-2), self_s(t-3), relu(t-5), y_s(t-6), out(t-6), exp(t)
        #   PE : selfT(t-2), aggs(t-2), WT(t-4), ytr(t-5)
        # Stage offsets are chosen so that EVERY cross-engine gate is
        # produced in a previous iteration -- each engine streams freely:
        #   SP  : loads(t)           POOL: nei c=2,3 (t), out(t-9)
        #   DVE : TT(t), reduce(t), softmax-tail(t-2) incl vtrans(t-2)
        #   ACT : copies(t-3), xt(t-5), relu(t-7), ys(t-9), exp(t)
        #   PE  : selfT+aggs(t-4), WT(t-6), ytr(t-8)

        @block.sync
        def _(sp):
            with nc.allow_non_contiguous_dma(reason="one-time preloads"):
                sp.dma_start(wt.ap(), w_d.ap().rearrange("j d -> d j")).then_inc(s_pre, 16)
                sp.dma_start(bias.ap(), b_d.ap()[:, None]).then_inc(s_pre, 16)
                sp.dma_start(self_all.ap().rearrange("p (t d) -> p t d", d=D),
                             self_cols).then_inc(s_pre, 16)
            for t in range(T + TAIL):
                if t < T:
                    wge(sp, s_scores, t - 2)  # rel[t%3] free: TT(t-3) done
                    wge(sp, s_agg, t - 5)     # nei[t%6] free: aggs(t-6) done
                    sp.dma_start(rel[t % 3].ap(),
                                 rel_rows[t * P:t * P + P, :]).then_inc(s_rel[t % 3], 16)
                    for c in (0, 1):
                        sp.dma_start(
                            nei[t % 6].ap()[32 * c:32 * c + 32, :].rearrange(
                                "p (j d) -> p j d", d=D),
                            nv_g[t, c],
                        ).then_inc(s_nei[t % 6], 16)

        # s_dve counts are tracked explicitly because head/tail iterations
        # skip stages.
        cnt = {"n": 0}
        c_tt, c_en, c_ts = {}, {}, {}

        @block.vector
        def _(v):
            wge(v, s_init, 9)
            wge(v, s_prea, 16)
            wge(v, s_prep, 16)
            for t in range(T + TAIL):
                if t < T:
                    i = t % 2
                    ut = user_all.ap()[:, D * (t % UB):D * (t % UB) + D]
                    wge(v, s_scores, t)       # prod WAR: reduce(t-1) done
                    wge(v, s_rel[t % 3], 16 * (t // 3 + 1))
                    wge(v, s_exp, t - 1)      # scores[i] WAR: exp(t-2) done
                    prod_v = prod.ap().rearrange("p (k d) -> p k d", k=K)
                    nc.vector.tensor_tensor(
                        prod_v, rel[t % 3].ap().rearrange("p (k d) -> p k d", k=K),
                        ut[:, None, :].to_broadcast((P, K, D)), ALU.mult
                    ).then_inc(s_dve, 1)
                    cnt["n"] += 1
                    c_tt[t] = cnt["n"]
                    wge(v, s_dve, c_tt[t])
                    nc.vector.reduce_sum(
                        scores[i].ap(), prod_v, axis=mybir.AxisListType.X
                    ).then_inc(s_scores, 1)
                u = t - 2
                if 0 <= u < T and u % 2 == 1:
                    pc = u // 2              # pair (2pc, 2pc+1)
                    nt = norm_all.ap()[:, 2 * K * pc:2 * K * pc + 2 * K]
                    wge(v, s_exp, u + 1)
                    nc.vector.reciprocal(recip.ap(), ssum[pc % 2].ap()).then_inc(s_dve, 1)
                    cnt["n"] += 1
                    nc.vector.tensor_tensor(
                        en.ap(), e_t[pc % 2].ap(), nt, ALU.mult
                    ).then_inc(s_dve, 1)
                    cnt["n"] += 1
                    c_en[pc] = cnt["n"]
                    wge(v, s_dve, c_en[pc])
                    wge(v, s_vt, pc)          # w2 WAR: vtrans(pc-1) done
                    nc.vector.tensor_tensor(
                        w2.ap().rearrange("p (b k) -> p b k", b=2),
                        en.ap().rearrange("p (b k) -> p b k", b=2),
                        recip.ap()[:, :, None].to_broadcast((P, 2, K)), ALU.mult
                    ).then_inc(s_dve, 1)
                    cnt["n"] += 1
                    c_ts[pc] = cnt["n"]
                    wge(v, s_dve, c_ts[pc])
                    wge(v, s_bdw, pc - 1)     # vtd[pc%2] WAR: copies(pc-2) done
                    nc.vector.transpose(vtd[pc % 2].ap(), w2.ap()).then_inc(s_vt, 1)

        @block.scalar
        def _(sc):
            with nc.allow_non_contiguous_dma(reason="one-time user preload"):
                sc.dma_start(user_all.ap().rearrange("p (m d) -> p m d", d=D),
                             user_cols[:, :UB, :]).then_inc(s_prea, 16)
            wge(sc, s_pre, 48)
            for t in range(T + TAIL):
                te = t - 1
                if 0 <= te < T:
                    if (te // 2 - 2) in c_en:
                        wge(sc, s_dve, c_en[te // 2 - 2])  # e/ssum buffer WAR
                    wge(sc, s_scores, te + 1)
                    h = te % 2
                    nc.scalar.activation(
                        e_t[(te // 2) % 2].ap()[:, K * h:K * h + K],
                        scores[te % 2].ap(), AF.Exp,
                        accum_out=ssum[(te // 2) % 2].ap()[:, h:h + 1]
                    ).then_inc(s_exp, 1)
                u2 = t - 3
                if 0 <= u2 < T and u2 % 2 == 1:
                    pc2 = u2 // 2
                    wge(sc, s_vt, pc2 + 1)
                    wge(sc, s_agg, 2 * pc2 - 2)  # bdw[pc2%2] WAR: aggs(pair-2)
                    bvp = bdw[pc2 % 2].ap().rearrange(
                        "p (b j c) -> p c b j", b=2, c=4)
                    vv = vtd[pc2 % 2].ap().rearrange("p (b j) -> p b j", b=2)
                    for c in range(4):
                        inst = nc.scalar.copy(
                            out=bvp[32 * c:32 * c + 32, c],
                            in_=vv[32 * c:32 * c + 32])
                    inst.then_inc(s_bdw, 1)
                u3 = t - 5
                if 0 <= u3 < T:
                    wge(sc, s_agg, u3 + 1)    # selfT+aggs(u3) in PSUM
                    wge(sc, s_wt, u3 - 1)     # xt_s[u3%2] WAR: WT(u3-2) done
                    nc.scalar.copy(xt_s[u3 % 2].ap(),
                                   agg_ps[u3 % 2].ap()).then_inc(s_xts, 1)
                x2 = t - 7
                if 0 <= x2 < T:
                    wge(sc, s_ytr, x2 - 1)    # yt_s[x2%2] WAR: ytr(x2-2) done
                    wge(sc, s_wt, x2 + 1)
                    nc.scalar.activation(
                        yt_s[x2 % 2].ap(), yt_ps[x2 % 2].ap(), AF.Relu,
                        bias=bias.ap()
                    ).then_inc(s_relu, 1)
                w2 = t - 9
                if 0 <= w2 < T:
                    wge(sc, s_out[w2 % 2], 16 * (w2 // 2))  # y_s WAR: out(w2-2)
                    wge(sc, s_ytr, w2 + 1)
                    nc.scalar.copy(y_s[w2 % 2].ap(),
                                   y_ps[w2 % 2].ap()).then_inc(s_ys, 1)


        @block.tensor
        def _(pe):
            wge(pe, s_init, 9)
            wge(pe, s_pre, 48)
            for t in range(T + TAIL):
                u = t - 4
                if 0 <= u < T:
                    st = self_all.ap()[:, D * u:D * u + D]
                    wge(pe, s_xts, u - 1)     # agg_ps[u%2] WAR: xt_s(u-2) done
                    nc.tensor.matmul(agg_ps[u % 2].ap(), st, perm.ap(),
                                     is_transpose=True, start=True, stop=False)
                    wge(pe, s_bdw, u // 2 + 1)
                    wge(pe, s_nei[u % 6], 32 * (u // 6 + 1))
                    wge(pe, s_nep[u % 6], 32 * (u // 6 + 1))
                    boff = P * (u % 2)
                    for j in range(K):
                        inst = nc.tensor.matmul(
                            agg_ps[u % 2].ap()[:, 4 * j:4 * j + 4],
                            nei[u % 6].ap()[:, D * j:D * j + D],
                            bdw[(u // 2) % 2].ap()[:, boff + 4 * j:boff + 4 * j + 4],
                            start=False, stop=(j == K - 1))
                    inst.then_inc(s_agg, 1)
                x3 = t - 6
                if 0 <= x3 < T:
                    wge(pe, s_xts, x3 + 1)
                    wge(pe, s_relu, x3 - 1)   # yt_ps[x3%2] WAR: relu(x3-2) done
                    nc.tensor.matmul(yt_ps[x3 % 2].ap(), wt.ap(),
                                     xt_s[x3 % 2].ap(),
                                     start=True, stop=True).then_inc(s_wt, 1)
                w = t - 8
                if 0 <= w < T:
                    wge(pe, s_relu, w + 1)
                    wge(pe, s_ys, w - 1)      # y_ps[w%2] WAR: y_s(w-2) done
                    nc.tensor.matmul(y_ps[w % 2].ap(), yt_s[w % 2].ap(),
                                     ident.ap()[:D, :D],
                                     is_transpose=True, start=True, stop=True
                                     ).then_inc(s_ytr, 1)

    return nc


# PE writes each 128-row tile with rows in (j,c) order: output row 4*j+c
# holds logical row 32*c+j.  Undo after the gather.
_ROWPERM = np.arange(P).reshape(32, 4).T.reshape(P)  # q(lr) = 4*(lr%32)+lr//32


def unpermute(out_rows):
    r = out_rows.reshape(-1, P, D)
    return r[:, _ROWPERM, :].reshape(out_rows.shape)


_NC_CACHE: dict = {}


def _get_nc(r_loc: int):
    if r_loc not in _NC_CACHE:
        _NC_CACHE[r_loc] = build_nc(r_loc)
    return _NC_CACHE[r_loc]


def kernel(self_vectors, neighbor_vectors, neighbor_relations, neighbor_norms,
           user_embeddings, W, b, trace: bool = False):
    import ml_dtypes

    from concourse.bass_utils import run_bass_kernel_spmd

    nc = _get_nc(R_LOC)
    nloc = R_LOC * K
    # bf16 inputs for the scores TT (DVE 2x mode) and the aggregation
    # matmuls (single-pass PE); halves their HBM traffic.  Verified
    # end-to-end rel_l2 impact: 1.1e-04.
    nv_bf = np.ascontiguousarray(neighbor_vectors).astype(ml_dtypes.bfloat16)
    nr_bf = np.ascontiguousarray(neighbor_relations).astype(ml_dtypes.bfloat16)
    user_bf = np.ascontiguousarray(user_embeddings).astype(ml_dtypes.bfloat16)
    nnk = (np.ascontiguousarray(neighbor_norms) * np.float32(1.0 / K)).astype(np.float32)
    in_maps = []
    for s in range(NCORES):
        r0 = s * R_LOC
        n0 = r0 * K
        in_maps.append({
            "self_v": np.ascontiguousarray(self_vectors[r0:r0 + R_LOC]),
            "nei_v": nv_bf[n0:n0 + nloc],
            "nei_r": nr_bf[n0:n0 + nloc],
            "nei_n": nnk[n0:n0 + nloc],
            "user": user_bf,
            "W": np.ascontiguousarray(W),
            "b": np.ascontiguousarray(b),
        })
    res = run_bass_kernel_spmd(nc, in_maps, core_ids=list(range(NCORES)),
                               trace=trace)
    out = np.concatenate([unpermute(res.results[s]["out"])
                          for s in range(NCORES)], axis=0)
    if trace:
        return out, res
    return out
